# revision 23
# baseline (speedup 1.0000x reference)
"""3x3 conv (im2col formulation) as 9 shifted matmuls on TRN2, data-parallel over batch.

Full inputs: x [32, 128, 56, 56] f32, w [1152, 256] f32 (row = c*9 + kh*3 + kw).
Full output: [32, 256, 56, 56] f32.

Each of the 8 cores processes 4 batch images; no collectives. Per core:
  - Operands run in fp16 (host-cast): full 1-cycle/row PE rate, FWL weight
    loads that hide under the matmul stream, fp32 PSUM accumulation;
    measured rel err ~3e-4.
  - x images DMA straight into h-padded [128(c), 58, 56] SBUF tiles
    (contiguous per-partition destination = max descriptor size; DMA is
    descriptor-latency-bound, ~800ns/descriptor/engine). Only rows 0/57
    are memset; horizontal taps handle w-padding via 55-wide matmuls into
    offset PSUM slices.
  - Per (image, out-channel half, 8-row band): 9 tap matmuls accumulate
    w_tap.T @ x_shifted into a [128(o), 8, 56] PSUM bank; DVE copies the
    band into a [128, 56, 56] SBUF image; bands stream to DRAM immediately,
    alternating between the two HWDGE rings (sync/scalar).
  - Image-0 load is split and ring-ordered so the first matmul only waits
    on the lower half + first weight half; bf16 warmup matmuls trip the PE
    HAM clock gate during the lead-in so the real stream starts at 2.4GHz.
"""

import numpy as np

import concourse.bass as bass  # noqa: F401  (registers AP types)
import concourse.mybir as mybir
import concourse.tile as tile
from concourse import bacc, bass_utils

B, C, H, W = 32, 128, 56, 56
COUT = 256
NCORES = 8
BPC = B // NCORES  # images per core
HP = H + 2
# tap order: dw=0 taps first (full width, carries the PSUM start flag)
TAPS = ([(dh, 0) for dh in (-1, 0, 1)]
        + [(dh, -1) for dh in (-1, 0, 1)]
        + [(dh, 1) for dh in (-1, 0, 1)])
HROWS = 8  # output rows per PSUM band
HT = H // HROWS  # bands per image
F32 = mybir.dt.float32
F32R = mybir.dt.float32r
BF16 = mybir.dt.bfloat16
MOV = mybir.dt.float16  # matmul operand dtype (fp16: full PE rate, FWL LDW)
MOV_NP = np.float16

_cached_nc = None


def _build():
    nc = bacc.Bacc(None, target_bir_lowering=False)
    x = nc.dram_tensor("x", [BPC, C, H, W], MOV, kind="ExternalInput")
    # host pre-arranges w as [oc_half, c, tap, 128] so each half DMAs with
    # fully contiguous per-partition chunks
    w = nc.dram_tensor("w", [2, C, 9, 128], MOV, kind="ExternalInput")
    out = nc.dram_tensor("out", [BPC, COUT, H, W], F32, kind="ExternalOutput")

    with tile.TileContext(nc) as tc:
        with (
            tc.tile_pool(name="wpool", bufs=1) as wpool,
            tc.tile_pool(name="xpool", bufs=2) as xpool,
            tc.tile_pool(name="opool", bufs=2) as opool,
            tc.tile_pool(name="pspool", bufs=8, space="PSUM") as pspool,
        ):
            # PE warmup: tiny matmuls with no data deps keep the PE busy
            # during the input DMA so HAM reaches K=8/8 before the real work.
            # Full-width warmup keeps PE duty-cycle high enough to trip the
            # HAM activity monitor (N=16 warmups run at ~27% duty and don't).
            NWARM = 12
            warm = wpool.tile([C, 448], BF16)
            nc.vector.memset(warm[:], 0.0)
            wpsum = pspool.tile([16, 448], F32, tag="pt", name="warm_psum")
            for i in range(NWARM):
                nc.tensor.matmul(wpsum[:], warm[:, :16], warm[:],
                                 start=(i == 0), stop=(i == NWARM - 1))

            # h-padded only ([C, 58, 56]): the input DMA destination is
            # fully contiguous per partition, so images load straight into
            # the compute tile — no staging, no pad copy. Horizontal taps
            # use 55-wide matmuls into offset PSUM slices instead.
            # Image 0 is split so bands 0-2 start after the lower half.
            HSPL = 28
            wbuf = wpool.tile([C, 2, 9, 128], MOV)
            xp0 = xpool.tile([C, HP, W], MOV, tag="xp", name="xp0")
            nc.sync.dma_start(xp0[:, 1 : HSPL + 1, :], x[0, :, :HSPL, :])
            nc.sync.dma_start(wbuf[:, 0], w[0])
            nc.sync.dma_start(xp0[:, HSPL + 1 : H + 1, :], x[0, :, HSPL:, :])
            nc.sync.dma_start(wbuf[:, 1], w[1])

            for b in range(BPC):
                if b == 0:
                    xp = xp0
                else:
                    xp = xpool.tile([C, HP, W], MOV, tag="xp", name=f"xp{b}")
                    nc.sync.dma_start(xp[:, 1 : H + 1, :], x[b])
                nc.vector.memset(xp[:, 0, :], 0.0)
                nc.vector.memset(xp[:, HP - 1, :], 0.0)

                for oc in range(COUT // 128):
                    oimg = opool.tile([128, H, W], F32, tag="oimg", name=f"oimg{b}_{oc}")
                    for ht in range(HT):
                        pt = pspool.tile(
                            [128, HROWS, W], F32, tag="pt", name=f"pt{b}_{oc}_{ht}"
                        )
                        for t, (dh, dw) in enumerate(TAPS):
                            kk = (dh + 1) * 3 + (dw + 1)
                            h0 = ht * HROWS + dh + 1
                            if dw == 0:
                                rhs = xp[:, h0 : h0 + HROWS, :]
                                dst = pt[:]
                            elif dw == -1:
                                rhs = xp[:, h0 : h0 + HROWS, 0 : W - 1]
                                dst = pt[:, :, 1:W]
                            else:
                                rhs = xp[:, h0 : h0 + HROWS, 1:W]
                                dst = pt[:, :, 0 : W - 1]
                            lhsT = wbuf[:, oc, kk, :]
                            nc.tensor.matmul(
                                dst, lhsT, rhs, start=(t == 0), stop=(t == 8)
                            )
                        last_band = b == BPC - 1 and oc == 1 and ht == HT - 1
                        if last_band:
                            # split the final band 6+2 so only a 2-row
                            # copy+DMA trails the last matmul
                            for part, (p0, rows) in enumerate([(0, 6), (6, 2)]):
                                r0 = ht * HROWS + p0
                                nc.vector.tensor_copy(
                                    out=oimg[:, r0 : r0 + rows, :],
                                    in_=pt[:, p0 : p0 + rows, :],
                                )
                                eng = nc.scalar if part % 2 else nc.sync
                                eng.dma_start(
                                    out[b, oc * 128 : (oc + 1) * 128, r0 : r0 + rows, :],
                                    oimg[:, r0 : r0 + rows, :],
                                )
                        else:
                            nc.vector.tensor_copy(
                                out=oimg[:, ht * HROWS : (ht + 1) * HROWS, :], in_=pt[:]
                            )
                            eng = nc.scalar if (b * 2 + oc * 7 + ht) % 2 else nc.sync
                            eng.dma_start(
                                out[b, oc * 128 : (oc + 1) * 128,
                                    ht * HROWS : (ht + 1) * HROWS, :],
                                oimg[:, ht * HROWS : (ht + 1) * HROWS, :],
                            )
    nc.compile()
    return nc


def _get_nc():
    global _cached_nc
    if _cached_nc is None:
        _cached_nc = _build()
    return _cached_nc


def run(x, w, trace=False, **spmd_kwargs):
    nc = _get_nc()
    x = np.ascontiguousarray(x, dtype=np.float32).astype(MOV_NP)
    w = np.asarray(w, dtype=np.float32)
    # [c*9, 256] -> [oc_half, c, tap, 128]
    w2 = np.ascontiguousarray(
        w.reshape(C, 9, 2, 128).transpose(2, 0, 1, 3)
    ).astype(MOV_NP)
    in_maps = [
        {"x": x[i * BPC : (i + 1) * BPC], "w": w2} for i in range(NCORES)
    ]
    res = bass_utils.run_bass_kernel_spmd(
        nc, in_maps, core_ids=list(range(NCORES)), trace=trace, **spmd_kwargs
    )
    full = np.concatenate([r["out"] for r in res.results], axis=0)
    return full, res


def kernel(x, w):
    return run(x, w)[0]
